# revision 25
# baseline (speedup 1.0000x reference)
# MoE routing kernel for Trainium2 (Bass/Tile), SPMD over 8 NeuronCores.
#
# Reference computation (B=4, T=2048, D=H=1024, V=8, L=4):
#   h      = gelu(einsum("btd,vdh->btvh", X, W1) + b1)
#   outs   = einsum("btvh,vhk->btvk", h, W2) + b2
#   w      = softmax(op_logits, axis=-1)            # [B, L, V]
#   result = einsum("blv,btvh->bth", w, outs) / L
#
# Strategy:
#   - Host: softmax + mean over L -> wbar[B, V]; fold b2 into a single
#     per-batch combined bias cbias[b] = sum_v wbar[b,v] * b2[v].
#   - Data parallel over tokens: core c owns tokens [c*1024, (c+1)*1024).
#     Each 1024-token shard lies inside a single batch row b, so wbar/cbias
#     are per-core constants (shipped as data => one SPMD program).
#   - Per core, per expert v:
#       MM1:  pre1^T[h, t] = sum_d W1[v][d, h]^T-free X^T[d, t]   (PE)
#       gelu: h_sb[h, t] = Gelu(pre1 + b1[v][h])                  (ACT, PSUM->SBUF)
#       MM2:  out[t, k]  = sum_h h_sb[h, t]-as-lhsT W2[v][h, k]   (PE)
#       acc:  out_acc[t, k] = wbar[v] * out + (cbias | out_acc)   (DVE)
#   - X is pre-transposed on host to [D, BT] so every matmul operand is
#     naturally contraction-major; no on-device transposes anywhere.
#
# Startup-latency design (the PE stream is otherwise at its physical
# floor of ~216ns per N=512 matmul):
#   - The DMA descriptor ring serializes transfers in issue order at
#     ~23.5GB/s per engine / ~375GB/s aggregate, and each dma_start
#     costs ~650ns of sequencer issue time. The baseline issued
#     b1/wbar/cbias/x (2.6MB, 12 dma_starts) ahead of w1[0], so the
#     first matmul could not start until ~19.4us.
#   - Fix: issue w1[0] immediately on the sync (SP) HWDGE ring while x
#     streams in parallel on the scalar (Activation) HWDGE ring; pack b1
#     into a [P, HC*V] layout (128B/partition, one small DMA instead of
#     1024 32B descriptors); push wbar/cbias behind w1[0].
#   - Expert 0's first token-half runs dc-OUTER across 8 interleaved
#     PSUM accumulation groups (one per hc), so matmuls start as soon as
#     the first (w1-chunk, x-chunk) pair lands and consume further pairs
#     as they arrive instead of waiting for the full 4MB.
#   - All PSUM tiles come from one 8-bank pool (same [P,512]xf32 slot),
#     so the startup phase may hold all 8 banks while steady state
#     rotates through them.

import os

import numpy as np
import ml_dtypes

import concourse.bass as bass
import concourse.mybir as mybir
import concourse.tile as tile
from concourse import bacc
from concourse.bass_utils import run_bass_kernel_spmd

N_CORES = 8
P = 128

_DT_MAP = {
    "bf16": mybir.dt.bfloat16,
    "f32r": mybir.dt.float32r,
    "f32": mybir.dt.float32,
}
_NP_DT_MAP = {
    "bf16": ml_dtypes.bfloat16,
    "f32r": np.float32,
    "f32": np.float32,
}


def build_moe_core_program(TC, D, H, V, mode="bf16", act="gelu"):
    """One NeuronCore's program: TC tokens, full V experts."""
    act_func = {
        "gelu": mybir.ActivationFunctionType.Gelu,
        "tanh": mybir.ActivationFunctionType.Tanh,  # sim-only (CoreSim lacks Gelu)
    }[act]
    DT = _DT_MAP[mode]
    f32 = mybir.dt.float32
    DC = D // P          # contraction chunks for MM1
    HC = H // P          # contraction chunks for MM2
    NT = min(512, TC)    # MM1 moving free dim (tokens)
    NK = min(512, H)     # MM2 moving free dim (output cols)
    TT = TC // P         # token tiles of 128
    NTH = TC // NT       # token halves

    # Bacc (not plain Bass): its finalize() runs generate_event_semaphores,
    # which splits multi-sem waits — TRN2 allows max 1 wait per instruction.
    nc = bacc.Bacc(trn_type="TRN2")
    x_t = nc.declare_dram_parameter("x_t", [D, TC], DT, isOutput=False)
    w1 = nc.declare_dram_parameter("w1", [V, D, H], DT, isOutput=False)
    w2 = nc.declare_dram_parameter("w2", [V, H, H], DT, isOutput=False)
    b1p = nc.declare_dram_parameter("b1p", [P, HC * V], f32, isOutput=False)
    wbar = nc.declare_dram_parameter("wbar", [P, V], f32, isOutput=False)
    cbias = nc.declare_dram_parameter("cbias", [P, H], f32, isOutput=False)
    out = nc.declare_dram_parameter("out", [TC, H], f32, isOutput=True)

    with tile.TileContext(nc) as tc:
        with (
            tc.tile_pool(name="const", bufs=1) as cpool,
            tc.tile_pool(name="w1p", bufs=2) as w1p,
            tc.tile_pool(name="w2p", bufs=2) as w2p,
            tc.tile_pool(name="hbuf", bufs=1) as hpool,
            tc.tile_pool(name="accp", bufs=1) as accp,
            tc.tile_pool(name="ps", bufs=8, space="PSUM") as ps,
        ):
            # (PE warm-up matmuls were tried here and reverted: the HAM
            # cold-clock window overlaps the DMA-bound startup trickle, so
            # warming the PE early just converts cold-rate compute into
            # pair-arrival stalls — measured net zero to slightly worse.)

            # per-tt output accumulators: each tt's store depends only on its
            # own tile, so final DMAs overlap the last expert's compute.
            out_tiles = [
                accp.tile([P, H], f32, tag=f"acc{tt}", name=f"acc{tt}")
                for tt in range(TT)
            ]
            out_r = out.rearrange("(tt p) k -> p tt k", p=P)

            # Expert 0's w1 tiles go on the sync ring FIRST (startup
            # critical); x tiles stream concurrently on the scalar
            # (Activation) HWDGE ring. Each ring admits ~512 descriptors
            # and drains all admitted transfers round-robin, so the first
            # usable (w1-dc, x-dc) pair completes when the whole first
            # wave does. Column-half transfers (1KB lines, 128 desc)
            # halve the first-wave byte count, and x-lo halves (the th0
            # operand) are admitted before all x-hi halves.
            w1_tiles0 = []
            w1_half_dmas = []
            b1_sb = None
            for dc in range(DC):
                w1t = w1p.tile([P, H], DT, tag=f"w1_{dc}", name=f"w1_{dc}")
                for half in range(2):
                    hs = slice(half * (H // 2), (half + 1) * (H // 2))
                    w1dma = nc.sync.dma_start(
                        out=w1t[:, hs], in_=w1[0, dc * P:(dc + 1) * P, hs]
                    )
                    w1_half_dmas.append(w1dma)
                w1_tiles0.append(w1t)
                if dc == 0:
                    # b1 packed [P, HC*V]: one 128B line per partition —
                    # a single small DMA instead of 1024 32B descriptors.
                    # Issued after w1-dc0 (it is only needed by the first
                    # activation, ~14us after the first matmul; placing it
                    # later was measured ~2us worse).
                    b1_sb = cpool.tile([P, HC, V], f32)
                    nc.sync.dma_start(out=b1_sb, in_=b1p[:])
            # x-lo in 512B-line quarters: DMA engines service active
            # transfers round-robin per descriptor, so halving x's
            # descriptor size shifts relative bandwidth toward the
            # startup-critical w1 halves on the sync ring.
            x_tiles = []
            for dc in range(DC):
                xt = cpool.tile([P, TC], DT, tag=f"x{dc}", name=f"x{dc}")
                for q in range(2):
                    qs = slice(q * (NT // 2), (q + 1) * (NT // 2))
                    nc.scalar.dma_start(
                        out=xt[:, qs], in_=x_t[dc * P:(dc + 1) * P, qs]
                    )
                x_tiles.append(xt)
            # x-hi halves (the th1 operand, needed only after th0's 14us of
            # compute) are held back until w1[0] is nearly done (dc6-hi):
            # the first one's issue waits on that, which blocks the
            # in-order act ring behind it, so the early startup waves stay
            # pure {w1, x-lo} and the w1 tail isn't displaced.
            for dc in range(DC):
                xhidma = nc.scalar.dma_start(
                    out=x_tiles[dc][:, NT:TC],
                    in_=x_t[dc * P:(dc + 1) * P, NT:TC],
                )
                if dc == 0:
                    # w1_half_dmas[13] = dc6's second half.
                    bass._add_dep_helper(
                        xhidma.ins, w1_half_dmas[13].ins, sync=True,
                        reason="keep x-hi out of the early startup DMA waves",
                    )
            # wbar is tiny (128x32B); issue behind w1[0] unchained.
            wbar_sb = cpool.tile([P, V], f32)
            nc.sync.dma_start(out=wbar_sb, in_=wbar[:])

            # Later weight blocks are chained (2MB granularity) with explicit
            # sync deps: each block's descriptors only enqueue after the
            # previous block's transfer completes, so they can't steal DMA
            # bandwidth from the startup-critical x + w1[0] transfers.
            prev_block = w1dma  # last DMA of the previous weight block

            def _dep_on_prev(dma):
                if prev_block is not None:
                    bass._add_dep_helper(
                        dma.ins, prev_block.ins, sync=True,
                        reason="serialize weight-block DMA issue",
                    )

            # cbias (512KB) is only needed by the first MM2 accumulate
            # (~40us in): chain it behind the last x-hi half so it can't
            # compete with the startup or th1 data. w2[0] then chains
            # behind cbias via prev_block.
            cbias_sb = cpool.tile([P, H], f32)
            cbdma = nc.sync.dma_start(out=cbias_sb, in_=cbias[:])
            bass._add_dep_helper(
                cbdma.ins, xhidma.ins, sync=True,
                reason="keep cbias out of the startup DMA waves",
            )
            prev_block = cbdma

            for v in range(V):
                if v == 0:
                    w1_tiles = w1_tiles0
                else:
                    # w1 per-dc tiles: chained behind the previous block.
                    w1_tiles = []
                    for dc in range(DC):
                        w1t = w1p.tile([P, H], DT, tag=f"w1_{dc}", name=f"w1_{dc}")
                        w1dma = nc.sync.dma_start(
                            out=w1t, in_=w1[v, dc * P:(dc + 1) * P, :]
                        )
                        _dep_on_prev(w1dma)
                        w1_tiles.append(w1t)
                    prev_block = w1dma
                h_sb = hpool.tile([P, HC, TC], DT, tag="h")

                if v == 0:
                    # Startup: dc-outer across 8 interleaved PSUM groups
                    # (one per hc) for the first token half — each matmul
                    # needs only (w1[0][dc], x[dc]), so the PE starts on
                    # the first arrived pair and tracks the DMA stream.
                    banks = [
                        ps.tile([P, NT], f32, tag="ps", name=f"pb{hc}")
                        for hc in range(HC)
                    ]
                    for dc in range(DC):
                        for hc in range(HC):
                            nc.tensor.matmul(
                                banks[hc],
                                w1_tiles[dc][:, hc * P:(hc + 1) * P],
                                x_tiles[dc][:, 0:NT],
                                start=(dc == 0),
                                stop=(dc == DC - 1),
                            )
                    for hc in range(HC):
                        nc.scalar.activation(
                            h_sb[:, hc, 0:NT],
                            banks[hc],
                            act_func,
                            bias=b1_sb[:, hc, v:v + 1],
                        )
                    rest_th = range(1, NTH)
                else:
                    rest_th = range(NTH)

                # MM1 + gelu: produce h-major activations h_sb[h, t]
                for hc in range(HC):
                    for th in rest_th:
                        p1 = ps.tile([P, NT], f32, tag="ps")
                        for dc in range(DC):
                            nc.tensor.matmul(
                                p1,
                                w1_tiles[dc][:, hc * P:(hc + 1) * P],
                                x_tiles[dc][:, th * NT:(th + 1) * NT],
                                start=(dc == 0),
                                stop=(dc == DC - 1),
                            )
                        nc.scalar.activation(
                            h_sb[:, hc, th * NT:(th + 1) * NT],
                            p1,
                            act_func,
                            bias=b1_sb[:, hc, v:v + 1],
                        )

                # w2 emitted after MM1 + chained, so its transfer can't steal
                # bandwidth from the startup-critical x/w1[0] loads.
                w2_sb = w2p.tile([P, HC, H], DT, tag="w2")
                w2dma = nc.sync.dma_start(
                    out=w2_sb, in_=w2[v].rearrange("(hc p) k -> p hc k", p=P)
                )
                _dep_on_prev(w2dma)
                prev_block = w2dma

                # MM2 + weighted accumulate into out_tiles[tt][t, k]
                for tt in range(TT):
                    for kc in range(H // NK):
                        p2 = ps.tile([P, NK], f32, tag="ps")
                        for hc in range(HC):
                            nc.tensor.matmul(
                                p2,
                                h_sb[:, hc, tt * P:(tt + 1) * P],
                                w2_sb[:, hc, kc * NK:(kc + 1) * NK],
                                start=(hc == 0),
                                stop=(hc == HC - 1),
                            )
                        if v == V - 1 and tt == TT - 1:
                            # The very last accumulates sit on the kernel's
                            # critical tail: split them into 256-col chunks
                            # so the final store is 128KB and starts sooner.
                            # (Putting any store on the act ring measured
                            # +90us — keep all output stores on sync.)
                            NQ = NK // 2
                            for q in range(2):
                                qsl = slice(kc * NK + q * NQ,
                                            kc * NK + (q + 1) * NQ)
                                nc.vector.scalar_tensor_tensor(
                                    out=out_tiles[tt][:, qsl],
                                    in0=p2[:, q * NQ:(q + 1) * NQ],
                                    scalar=wbar_sb[:, v:v + 1],
                                    in1=out_tiles[tt][:, qsl],
                                    op0=mybir.AluOpType.mult,
                                    op1=mybir.AluOpType.add,
                                )
                                nc.sync.dma_start(
                                    out=out_r[:, tt, qsl],
                                    in_=out_tiles[tt][:, qsl],
                                )
                            continue
                        ksl = slice(kc * NK, (kc + 1) * NK)
                        in1 = cbias_sb[:, ksl] if v == 0 else out_tiles[tt][:, ksl]
                        nc.vector.scalar_tensor_tensor(
                            out=out_tiles[tt][:, ksl],
                            in0=p2,
                            scalar=wbar_sb[:, v:v + 1],
                            in1=in1,
                            op0=mybir.AluOpType.mult,
                            op1=mybir.AluOpType.add,
                        )
                        if v == V - 1:
                            # store each kc half as soon as it is final —
                            # halves the post-stream store tail.
                            nc.sync.dma_start(
                                out=out_r[:, tt, ksl], in_=out_tiles[tt][:, ksl]
                            )
    nc.finalize()  # Bacc: runs wait-splitting + reg alloc passes
    return nc


_prog_cache = {}


def _get_program(mode, TC, D, H, V):
    key = (mode, TC, D, H, V)
    if key not in _prog_cache:
        _prog_cache[key] = build_moe_core_program(TC, D, H, V, mode=mode)
    return _prog_cache[key]


def host_prep(op_logits, token_feats, W1, b1, W2, b2, mode):
    """Shared host-side preprocessing: softmax folding, transpose, cast, shard."""
    op_logits = np.asarray(op_logits, dtype=np.float32)
    token_feats = np.asarray(token_feats, dtype=np.float32)
    W1 = np.asarray(W1, dtype=np.float32)
    b1 = np.asarray(b1, dtype=np.float32)
    W2 = np.asarray(W2, dtype=np.float32)
    b2 = np.asarray(b2, dtype=np.float32)

    B, T, D = token_feats.shape
    V, _, H = W1.shape
    BT = B * T
    TC = BT // N_CORES
    HC = H // P

    lg = op_logits.astype(np.float64)
    e = np.exp(lg - lg.max(axis=-1, keepdims=True))
    w = e / e.sum(axis=-1, keepdims=True)
    wbar = w.mean(axis=1)                       # [B, V], includes the 1/L
    cbias = wbar @ b2.astype(np.float64)        # [B, H]

    np_dt = _NP_DT_MAP[mode]
    x_t = np.ascontiguousarray(token_feats.reshape(BT, D).T).astype(np_dt)
    w1c = np.ascontiguousarray(W1.astype(np_dt))
    w2c = np.ascontiguousarray(W2.astype(np_dt))
    # b1 packed [P, HC*V]: b1p[p, hc*V+v] = b1[v, hc*128+p] — one 128B
    # contiguous line per partition.
    b1p = np.ascontiguousarray(
        b1.T.reshape(HC, P, V).transpose(1, 0, 2).reshape(P, HC * V)
    ).astype(np.float32)

    in_maps = []
    for c in range(N_CORES):
        bc = (c * TC) // T
        in_maps.append({
            "x_t": np.ascontiguousarray(x_t[:, c * TC:(c + 1) * TC]),
            "w1": w1c,
            "w2": w2c,
            "b1p": b1p,
            "wbar": np.ascontiguousarray(
                np.broadcast_to(wbar[bc].astype(np.float32), (P, V))
            ),
            "cbias": np.ascontiguousarray(
                np.broadcast_to(cbias[bc].astype(np.float32), (P, H))
            ),
        })
    return in_maps, (B, T, D, H, V, TC)


LAST_RESULTS = None


def kernel(op_logits, token_feats, W1, b1, W2, b2):
    global LAST_RESULTS
    mode = os.environ.get("MOE_DTYPE", "bf16")
    in_maps, (B, T, D, H, V, TC) = host_prep(
        op_logits, token_feats, W1, b1, W2, b2, mode
    )
    nc = _get_program(mode, TC, D, H, V)
    res = run_bass_kernel_spmd(
        nc,
        in_maps,
        list(range(N_CORES)),
        trace=os.environ.get("MOE_TRACE", "0") == "1",
    )
    LAST_RESULTS = res
    outs = [res.results[c]["out"] for c in range(N_CORES)]
    return np.concatenate(outs, axis=0).reshape(B, T, H).astype(np.float32)


# revision 26
# speedup vs baseline: 1.0077x; 1.0077x over previous
# MoE routing kernel for Trainium2 (Bass/Tile), SPMD over 8 NeuronCores.
#
# Reference computation (B=4, T=2048, D=H=1024, V=8, L=4):
#   h      = gelu(einsum("btd,vdh->btvh", X, W1) + b1)
#   outs   = einsum("btvh,vhk->btvk", h, W2) + b2
#   w      = softmax(op_logits, axis=-1)            # [B, L, V]
#   result = einsum("blv,btvh->bth", w, outs) / L
#
# Strategy:
#   - Host: softmax + mean over L -> wbar[B, V]; fold b2 into a single
#     per-batch combined bias cbias[b] = sum_v wbar[b,v] * b2[v].
#   - Data parallel over tokens: core c owns tokens [c*1024, (c+1)*1024).
#     Each 1024-token shard lies inside a single batch row b, so wbar/cbias
#     are per-core constants (shipped as data => one SPMD program).
#   - Per core, per expert v:
#       MM1:  pre1^T[h, t] = sum_d W1[v][d, h]^T-free X^T[d, t]   (PE)
#       gelu: h_sb[h, t] = Gelu(pre1 + b1[v][h])                  (ACT, PSUM->SBUF)
#       MM2:  out[t, k]  = sum_h h_sb[h, t]-as-lhsT W2[v][h, k]   (PE)
#       acc:  out_acc[t, k] = wbar[v] * out + (cbias | out_acc)   (DVE)
#   - X is pre-transposed on host to [D, BT] so every matmul operand is
#     naturally contraction-major; no on-device transposes anywhere.
#
# Startup-latency design (the PE stream is otherwise at its physical
# floor of ~216ns per N=512 matmul):
#   - The DMA descriptor ring serializes transfers in issue order at
#     ~23.5GB/s per engine / ~375GB/s aggregate, and each dma_start
#     costs ~650ns of sequencer issue time. The baseline issued
#     b1/wbar/cbias/x (2.6MB, 12 dma_starts) ahead of w1[0], so the
#     first matmul could not start until ~19.4us.
#   - Fix: issue w1[0] immediately on the sync (SP) HWDGE ring while x
#     streams in parallel on the scalar (Activation) HWDGE ring; pack b1
#     into a [P, HC*V] layout (128B/partition, one small DMA instead of
#     1024 32B descriptors); push wbar/cbias behind w1[0].
#   - Expert 0's first token-half runs dc-OUTER across 8 interleaved
#     PSUM accumulation groups (one per hc), so matmuls start as soon as
#     the first (w1-chunk, x-chunk) pair lands and consume further pairs
#     as they arrive instead of waiting for the full 4MB.
#   - All PSUM tiles come from one 8-bank pool (same [P,512]xf32 slot),
#     so the startup phase may hold all 8 banks while steady state
#     rotates through them.

import os

import numpy as np
import ml_dtypes

import concourse.bass as bass
import concourse.mybir as mybir
import concourse.tile as tile
from concourse import bacc
from concourse.bass_utils import run_bass_kernel_spmd

N_CORES = 8
P = 128

_DT_MAP = {
    "bf16": mybir.dt.bfloat16,
    "f32r": mybir.dt.float32r,
    "f32": mybir.dt.float32,
}
_NP_DT_MAP = {
    "bf16": ml_dtypes.bfloat16,
    "f32r": np.float32,
    "f32": np.float32,
}


def build_moe_core_program(TC, D, H, V, mode="bf16", act="gelu"):
    """One NeuronCore's program: TC tokens, full V experts."""
    act_func = {
        "gelu": mybir.ActivationFunctionType.Gelu,
        "tanh": mybir.ActivationFunctionType.Tanh,  # sim-only (CoreSim lacks Gelu)
    }[act]
    DT = _DT_MAP[mode]
    f32 = mybir.dt.float32
    DC = D // P          # contraction chunks for MM1
    HC = H // P          # contraction chunks for MM2
    NT = min(512, TC)    # MM1 moving free dim (tokens)
    NK = min(512, H)     # MM2 moving free dim (output cols)
    TT = TC // P         # token tiles of 128
    NTH = TC // NT       # token halves

    # Bacc (not plain Bass): its finalize() runs generate_event_semaphores,
    # which splits multi-sem waits — TRN2 allows max 1 wait per instruction.
    nc = bacc.Bacc(trn_type="TRN2")
    x_t = nc.declare_dram_parameter("x_t", [D, TC], DT, isOutput=False)
    w1 = nc.declare_dram_parameter("w1", [V, D, H], DT, isOutput=False)
    w2 = nc.declare_dram_parameter("w2", [V, H, H], DT, isOutput=False)
    b1p = nc.declare_dram_parameter("b1p", [P, HC * V], f32, isOutput=False)
    wbar = nc.declare_dram_parameter("wbar", [P, V], f32, isOutput=False)
    cbias = nc.declare_dram_parameter("cbias", [P, H], f32, isOutput=False)
    out = nc.declare_dram_parameter("out", [TC, H], f32, isOutput=True)

    with tile.TileContext(nc) as tc:
        with (
            tc.tile_pool(name="const", bufs=1) as cpool,
            tc.tile_pool(name="w1p", bufs=2) as w1p,
            tc.tile_pool(name="w2p", bufs=2) as w2p,
            tc.tile_pool(name="hbuf", bufs=1) as hpool,
            tc.tile_pool(name="accp", bufs=1) as accp,
            tc.tile_pool(name="ps", bufs=8, space="PSUM") as ps,
        ):
            # (PE warm-up matmuls were tried here and reverted: the HAM
            # cold-clock window overlaps the DMA-bound startup trickle, so
            # warming the PE early just converts cold-rate compute into
            # pair-arrival stalls — measured net zero to slightly worse.)

            # per-tt output accumulators: each tt's store depends only on its
            # own tile, so final DMAs overlap the last expert's compute.
            out_tiles = [
                accp.tile([P, H], f32, tag=f"acc{tt}", name=f"acc{tt}")
                for tt in range(TT)
            ]
            out_r = out.rearrange("(tt p) k -> p tt k", p=P)

            # Expert 0's w1 tiles go on the sync ring FIRST (startup
            # critical); x tiles stream concurrently on the scalar
            # (Activation) HWDGE ring. Each ring admits ~512 descriptors
            # and drains all admitted transfers round-robin, so the first
            # usable (w1-dc, x-dc) pair completes when the whole first
            # wave does. Column-half transfers (1KB lines, 128 desc)
            # halve the first-wave byte count, and x-lo halves (the th0
            # operand) are admitted before all x-hi halves.
            w1_tiles0 = []
            w1_half_dmas = []
            b1_sb = None
            for dc in range(DC):
                w1t = w1p.tile([P, H], DT, tag=f"w1_{dc}", name=f"w1_{dc}")
                for half in range(2):
                    hs = slice(half * (H // 2), (half + 1) * (H // 2))
                    w1dma = nc.sync.dma_start(
                        out=w1t[:, hs], in_=w1[0, dc * P:(dc + 1) * P, hs]
                    )
                    w1_half_dmas.append(w1dma)
                w1_tiles0.append(w1t)
                if dc == 0:
                    # b1 packed [P, HC*V]: one 128B line per partition —
                    # a single small DMA instead of 1024 32B descriptors.
                    # Issued after w1-dc0 (it is only needed by the first
                    # activation, ~14us after the first matmul; placing it
                    # later was measured ~2us worse).
                    b1_sb = cpool.tile([P, HC, V], f32)
                    nc.sync.dma_start(out=b1_sb, in_=b1p[:])
            x_tiles = []
            for dc in range(DC):
                xt = cpool.tile([P, TC], DT, tag=f"x{dc}", name=f"x{dc}")
                nc.scalar.dma_start(
                    out=xt[:, 0:NT], in_=x_t[dc * P:(dc + 1) * P, 0:NT]
                )
                x_tiles.append(xt)
            # x-hi halves (the th1 operand, needed only after th0's 14us of
            # compute) are held back until w1[0] is nearly done (dc6-hi):
            # the first one's issue waits on that, which blocks the
            # in-order act ring behind it, so the early startup waves stay
            # pure {w1, x-lo} and the w1 tail isn't displaced.
            for dc in range(DC):
                xhidma = nc.scalar.dma_start(
                    out=x_tiles[dc][:, NT:TC],
                    in_=x_t[dc * P:(dc + 1) * P, NT:TC],
                )
                if dc == 0:
                    # w1_half_dmas[13] = dc6's second half.
                    bass._add_dep_helper(
                        xhidma.ins, w1_half_dmas[13].ins, sync=True,
                        reason="keep x-hi out of the early startup DMA waves",
                    )
            # wbar is tiny (128x32B); issue behind w1[0] unchained.
            wbar_sb = cpool.tile([P, V], f32)
            nc.sync.dma_start(out=wbar_sb, in_=wbar[:])

            # Later weight blocks are chained (2MB granularity) with explicit
            # sync deps: each block's descriptors only enqueue after the
            # previous block's transfer completes, so they can't steal DMA
            # bandwidth from the startup-critical x + w1[0] transfers.
            prev_block = w1dma  # last DMA of the previous weight block

            def _dep_on_prev(dma):
                if prev_block is not None:
                    bass._add_dep_helper(
                        dma.ins, prev_block.ins, sync=True,
                        reason="serialize weight-block DMA issue",
                    )

            # cbias (512KB) is only needed by the first MM2 accumulate
            # (~40us in): chain it behind the last x-hi half so it can't
            # compete with the startup or th1 data. w2[0] then chains
            # behind cbias via prev_block.
            cbias_sb = cpool.tile([P, H], f32)
            cbdma = nc.sync.dma_start(out=cbias_sb, in_=cbias[:])
            bass._add_dep_helper(
                cbdma.ins, xhidma.ins, sync=True,
                reason="keep cbias out of the startup DMA waves",
            )
            prev_block = cbdma

            for v in range(V):
                if v == 0:
                    w1_tiles = w1_tiles0
                else:
                    # w1 per-dc tiles: chained behind the previous block.
                    w1_tiles = []
                    for dc in range(DC):
                        w1t = w1p.tile([P, H], DT, tag=f"w1_{dc}", name=f"w1_{dc}")
                        w1dma = nc.sync.dma_start(
                            out=w1t, in_=w1[v, dc * P:(dc + 1) * P, :]
                        )
                        _dep_on_prev(w1dma)
                        w1_tiles.append(w1t)
                    prev_block = w1dma
                h_sb = hpool.tile([P, HC, TC], DT, tag="h")

                if v == 0:
                    # Startup: dc-outer across 8 interleaved PSUM groups
                    # (one per hc) for the first token half — each matmul
                    # needs only (w1[0][dc], x[dc]), so the PE starts on
                    # the first arrived pair and tracks the DMA stream.
                    banks = [
                        ps.tile([P, NT], f32, tag="ps", name=f"pb{hc}")
                        for hc in range(HC)
                    ]
                    for dc in range(DC):
                        for hc in range(HC):
                            nc.tensor.matmul(
                                banks[hc],
                                w1_tiles[dc][:, hc * P:(hc + 1) * P],
                                x_tiles[dc][:, 0:NT],
                                start=(dc == 0),
                                stop=(dc == DC - 1),
                            )
                    for hc in range(HC):
                        nc.scalar.activation(
                            h_sb[:, hc, 0:NT],
                            banks[hc],
                            act_func,
                            bias=b1_sb[:, hc, v:v + 1],
                        )
                    rest_th = range(1, NTH)
                else:
                    rest_th = range(NTH)

                # MM1 + gelu: produce h-major activations h_sb[h, t]
                for hc in range(HC):
                    for th in rest_th:
                        p1 = ps.tile([P, NT], f32, tag="ps")
                        for dc in range(DC):
                            nc.tensor.matmul(
                                p1,
                                w1_tiles[dc][:, hc * P:(hc + 1) * P],
                                x_tiles[dc][:, th * NT:(th + 1) * NT],
                                start=(dc == 0),
                                stop=(dc == DC - 1),
                            )
                        nc.scalar.activation(
                            h_sb[:, hc, th * NT:(th + 1) * NT],
                            p1,
                            act_func,
                            bias=b1_sb[:, hc, v:v + 1],
                        )

                # w2 emitted after MM1 + chained, so its transfer can't steal
                # bandwidth from the startup-critical x/w1[0] loads.
                w2_sb = w2p.tile([P, HC, H], DT, tag="w2")
                w2dma = nc.sync.dma_start(
                    out=w2_sb, in_=w2[v].rearrange("(hc p) k -> p hc k", p=P)
                )
                _dep_on_prev(w2dma)
                prev_block = w2dma

                # MM2 + weighted accumulate into out_tiles[tt][t, k]
                for tt in range(TT):
                    for kc in range(H // NK):
                        p2 = ps.tile([P, NK], f32, tag="ps")
                        for hc in range(HC):
                            nc.tensor.matmul(
                                p2,
                                h_sb[:, hc, tt * P:(tt + 1) * P],
                                w2_sb[:, hc, kc * NK:(kc + 1) * NK],
                                start=(hc == 0),
                                stop=(hc == HC - 1),
                            )
                        if v == V - 1 and tt == TT - 1:
                            # The very last accumulates sit on the kernel's
                            # critical tail: split them into 256-col chunks
                            # so the final store is 128KB and starts sooner.
                            # (Putting any store on the act ring measured
                            # +90us — keep all output stores on sync.)
                            NQ = NK // 2
                            for q in range(2):
                                qsl = slice(kc * NK + q * NQ,
                                            kc * NK + (q + 1) * NQ)
                                nc.vector.scalar_tensor_tensor(
                                    out=out_tiles[tt][:, qsl],
                                    in0=p2[:, q * NQ:(q + 1) * NQ],
                                    scalar=wbar_sb[:, v:v + 1],
                                    in1=out_tiles[tt][:, qsl],
                                    op0=mybir.AluOpType.mult,
                                    op1=mybir.AluOpType.add,
                                )
                                nc.sync.dma_start(
                                    out=out_r[:, tt, qsl],
                                    in_=out_tiles[tt][:, qsl],
                                )
                            continue
                        ksl = slice(kc * NK, (kc + 1) * NK)
                        in1 = cbias_sb[:, ksl] if v == 0 else out_tiles[tt][:, ksl]
                        nc.vector.scalar_tensor_tensor(
                            out=out_tiles[tt][:, ksl],
                            in0=p2,
                            scalar=wbar_sb[:, v:v + 1],
                            in1=in1,
                            op0=mybir.AluOpType.mult,
                            op1=mybir.AluOpType.add,
                        )
                        if v == V - 1:
                            # store each kc half as soon as it is final —
                            # halves the post-stream store tail.
                            nc.sync.dma_start(
                                out=out_r[:, tt, ksl], in_=out_tiles[tt][:, ksl]
                            )
    nc.finalize()  # Bacc: runs wait-splitting + reg alloc passes
    return nc


_prog_cache = {}


def _get_program(mode, TC, D, H, V):
    key = (mode, TC, D, H, V)
    if key not in _prog_cache:
        _prog_cache[key] = build_moe_core_program(TC, D, H, V, mode=mode)
    return _prog_cache[key]


def host_prep(op_logits, token_feats, W1, b1, W2, b2, mode):
    """Shared host-side preprocessing: softmax folding, transpose, cast, shard."""
    op_logits = np.asarray(op_logits, dtype=np.float32)
    token_feats = np.asarray(token_feats, dtype=np.float32)
    W1 = np.asarray(W1, dtype=np.float32)
    b1 = np.asarray(b1, dtype=np.float32)
    W2 = np.asarray(W2, dtype=np.float32)
    b2 = np.asarray(b2, dtype=np.float32)

    B, T, D = token_feats.shape
    V, _, H = W1.shape
    BT = B * T
    TC = BT // N_CORES
    HC = H // P

    lg = op_logits.astype(np.float64)
    e = np.exp(lg - lg.max(axis=-1, keepdims=True))
    w = e / e.sum(axis=-1, keepdims=True)
    wbar = w.mean(axis=1)                       # [B, V], includes the 1/L
    cbias = wbar @ b2.astype(np.float64)        # [B, H]

    np_dt = _NP_DT_MAP[mode]
    x_t = np.ascontiguousarray(token_feats.reshape(BT, D).T).astype(np_dt)
    w1c = np.ascontiguousarray(W1.astype(np_dt))
    w2c = np.ascontiguousarray(W2.astype(np_dt))
    # b1 packed [P, HC*V]: b1p[p, hc*V+v] = b1[v, hc*128+p] — one 128B
    # contiguous line per partition.
    b1p = np.ascontiguousarray(
        b1.T.reshape(HC, P, V).transpose(1, 0, 2).reshape(P, HC * V)
    ).astype(np.float32)

    in_maps = []
    for c in range(N_CORES):
        bc = (c * TC) // T
        in_maps.append({
            "x_t": np.ascontiguousarray(x_t[:, c * TC:(c + 1) * TC]),
            "w1": w1c,
            "w2": w2c,
            "b1p": b1p,
            "wbar": np.ascontiguousarray(
                np.broadcast_to(wbar[bc].astype(np.float32), (P, V))
            ),
            "cbias": np.ascontiguousarray(
                np.broadcast_to(cbias[bc].astype(np.float32), (P, H))
            ),
        })
    return in_maps, (B, T, D, H, V, TC)


LAST_RESULTS = None


def kernel(op_logits, token_feats, W1, b1, W2, b2):
    global LAST_RESULTS
    mode = os.environ.get("MOE_DTYPE", "bf16")
    in_maps, (B, T, D, H, V, TC) = host_prep(
        op_logits, token_feats, W1, b1, W2, b2, mode
    )
    nc = _get_program(mode, TC, D, H, V)
    res = run_bass_kernel_spmd(
        nc,
        in_maps,
        list(range(N_CORES)),
        trace=os.environ.get("MOE_TRACE", "0") == "1",
    )
    LAST_RESULTS = res
    outs = [res.results[c]["out"] for c in range(N_CORES)]
    return np.concatenate(outs, axis=0).reshape(B, T, H).astype(np.float32)
